# revision 23
# baseline (speedup 1.0000x reference)
"""GCN layer relu(GCNConv(x, edge_index)) on 8 Trainium2 NeuronCores.

Math (PyG GCNConv with self-loops, symmetric norm, zero-init bias):
    deg[v]  = 1 + in-degree(v)
    s       = deg ** -0.5
    out[d]  = relu(s[d] * (sum_{e: dst(e)=d} s[src_e] * (x[src_e] @ W)) + b)
with the self-loop folded in as a regular edge d -> d.

Two chained SPMD programs ("transform first"), host does index work only:

  pass 1  h' = s[n] * (x[n] @ W)  for each core's 12500 nodes
          (reads x fp16 [128, 12544] transposed, writes h' fp16 [*, 32])
  host    gathers the per-edge message table he = h'[src_slot] into a
          degree-sorted padded ELL layout (pure fancy-indexing, no FP)
  pass 2  per dst tile of 128 nodes: segment-sum the K slot-messages,
          scale by s[dst], add bias, relu.

The segment-sum runs on the TENSOR engine: 4 slots at a time land in 4
PSUM lanes via identity matmuls (out[128n, 4*32] += I @ he-slice), since
DVE tensor_reduce only has a 1x uop (~56us for the 7M elements) while PE
accumulate-copies hide under the DMA stream.  Lanes fold on DVE as two
k=2 reductions (every lane-half is guaranteed written for K>=4, so no
PSUM-stale-read hazard for odd pair counts).  The dest scale + relu ride
the otherwise-idle ACT engine (per-tile scale is a [P,1] activation
scale); the bias is pre-added as b*sqrt(deg) before scaling.

Messages are 64 B/slot (fp16 x 32 features) instead of the 256 B/slot a
transform-last design needs, cutting the dominant HBM stream 4x.
Indirect DMA stays off the table (one runtime offset per partition per
instruction, ~1us); replicated-but-sequential streams win on TRN2.
"""

import math
import numpy as np

import concourse.bass as bass
import concourse.bacc as bacc
import concourse.mybir as mybir
import concourse.tile as tile
from concourse import bass_utils

# ---------------------------------------------------------------- config ---
P = 128            # partitions
D_IN = 128
D_OUT = 32
N = 100000         # nodes
E = 1600000        # edges
NCORES = 8

NPC = N // NCORES              # 12500 nodes per core
TPC = math.ceil(NPC / P)       # 98 node tiles per core
NPOS = TPC * P                 # 12544 padded positions per core
NPAD0 = NPOS - NPC             # 44 pad positions (front, degree 0)
NV = NCORES * NPOS             # padded global positions

CHUNK1 = 14                    # pass-1 tiles per psum group
HALF1 = 49                     # pass-1 tiles per x DMA
SPLIT1 = 56                    # pass-1 h1 flush boundary (tile index)
MAXC2 = 16                    # pass-2 max tiles per he chunk
PGRP2 = 4                      # pass-2 tiles per psum bank

F16 = mybir.dt.float16
F32 = mybir.dt.float32


# ------------------------------------------------------------- host prep ---
def host_prep(x, edge_index):
    """Index bookkeeping only: shard, degree-sort, build the slot table."""
    src = np.asarray(edge_index[0]).astype(np.int64)
    dst = np.asarray(edge_index[1]).astype(np.int64)
    deg = np.bincount(dst, minlength=N).astype(np.int64) + 1   # + self loop

    # Per-core degree sort (ascending); pads sit in front with slot-deg 0.
    node_of_pos = np.full(NV, -1, dtype=np.int64)
    for c in range(NCORES):
        lo = c * NPC
        order = np.argsort(deg[lo:lo + NPC], kind="stable")
        node_of_pos[c * NPOS + NPAD0 + np.arange(NPC)] = lo + order
    valid = node_of_pos >= 0
    pos_of_node = np.empty(N, dtype=np.int64)
    pos_of_node[node_of_pos[valid]] = np.nonzero(valid)[0]

    sdeg = np.zeros(NV, dtype=np.int64)
    sdeg[valid] = deg[node_of_pos[valid]]

    # Shared per-tile slot count (SPMD: one program), padded even so every
    # tile is whole 64-column matmuls; >=4 so both psum lane-halves are
    # written by at least one 128-column matmul.
    ktile = sdeg.reshape(NCORES, TPC, P).max(axis=(0, 2))
    ktile = (np.maximum(ktile, 4) + 1) // 2 * 2
    offs = np.concatenate([[0], np.cumsum(ktile)]).astype(np.int64)
    totk = int(offs[-1])

    # slot source table: src_slot[core][p, slot] = source node (-1 pad).
    # Slot offs[t]+0 of node (t,p) is its self loop.
    src_slot = np.full((NCORES, P, totk), -1, dtype=np.int64)
    vreal = np.nonzero(valid)[0]
    src_slot[vreal // NPOS, vreal % P, offs[(vreal % NPOS) // P]] = (
        node_of_pos[vreal]
    )
    key = pos_of_node[dst]
    es = np.argsort(key, kind="stable")
    key_s = key[es]
    newrun = np.ones(E, dtype=bool)
    newrun[1:] = key_s[1:] != key_s[:-1]
    run_start = np.maximum.accumulate(np.where(newrun, np.arange(E), 0))
    kwith = np.arange(E) - run_start + 1
    src_slot[key_s // NPOS, key_s % P, offs[(key_s % NPOS) // P] + kwith] = (
        src[es]
    )

    # Per-position degree table (pass-1 source scale == pass-2 dest scale).
    degp = np.maximum(sdeg, 1).astype(np.float16)        # [NV]
    degp = degp.reshape(NCORES, TPC, P).transpose(0, 2, 1)  # [C, P, TPC]

    # Pass-1 x shard, transposed: column q = t*128+p holds x[node(q)].
    x16 = np.concatenate(
        [np.asarray(x).astype(np.float16), np.zeros((1, D_IN), np.float16)]
    )
    nop = np.where(valid, node_of_pos, N).reshape(NCORES, NPOS)
    xt = np.ascontiguousarray(x16[nop].transpose(0, 2, 1))  # [C, 128, NPOS]

    return xt, degp, src_slot, ktile, offs, totk, node_of_pos


# -------------------------------------------------------- pass-1 builder ---
def build_pass1():
    nc = bacc.Bacc(None, num_devices=NCORES)
    xt = nc.dram_tensor("xt", [P, NPOS], F16, kind="ExternalInput")
    deg1 = nc.dram_tensor("deg1", [P, TPC], F16, kind="ExternalInput")
    w = nc.dram_tensor("w", [P, D_OUT], F16, kind="ExternalInput")
    h1 = nc.dram_tensor("h1", [P, TPC * D_OUT], F16, kind="ExternalOutput")

    with tile.TileContext(nc) as tc:
        with (
            tc.tile_pool(name="const", bufs=1) as cpool,
            tc.tile_pool(name="psum", bufs=4, space="PSUM") as ppool,
        ):
            w_sb = cpool.tile([P, D_OUT], F16)
            deg_sb = cpool.tile([P, TPC], F16)
            rtmp = cpool.tile([P, TPC], F32)
            s1 = cpool.tile([P, TPC], F32)
            xa = cpool.tile([P, HALF1 * P], F16)
            xb = cpool.tile([P, (TPC - HALF1) * P], F16)
            h1a = cpool.tile([P, SPLIT1 * D_OUT], F16)
            h1b = cpool.tile([P, (TPC - SPLIT1) * D_OUT], F16)

            nc.sync.dma_start(out=w_sb[:], in_=w[:, :])
            nc.sync.dma_start(out=deg_sb[:], in_=deg1[:, :])
            nc.vector.reciprocal(out=rtmp[:], in_=deg_sb[:])
            nc.scalar.sqrt(out=s1[:], in_=rtmp[:])
            nc.sync.dma_start(out=xa[:], in_=xt[:, :HALF1 * P])
            nc.sync.dma_start(out=xb[:], in_=xt[:, HALF1 * P:])

            for c in range(math.ceil(TPC / CHUNK1)):
                t0, t1 = c * CHUNK1, min((c + 1) * CHUNK1, TPC)
                g = t1 - t0
                ps = ppool.tile([P, CHUNK1 * D_OUT], F32, tag="ps")
                for j, t in enumerate(range(t0, t1)):
                    xsrc = (
                        xa[:, t * P:(t + 1) * P] if t < HALF1
                        else xb[:, (t - HALF1) * P:(t - HALF1 + 1) * P]
                    )
                    nc.tensor.matmul(
                        out=ps[:, j * D_OUT:(j + 1) * D_OUT],
                        lhsT=xsrc,
                        rhs=w_sb[:],
                        start=(j == 0),
                        stop=(j == g - 1),
                    )
                if t1 <= SPLIT1:
                    hdst = h1a[:, t0 * D_OUT:t1 * D_OUT]
                else:
                    hdst = h1b[:, (t0 - SPLIT1) * D_OUT:(t1 - SPLIT1) * D_OUT]
                nc.vector.tensor_tensor(
                    out=hdst.rearrange("p (t f) -> p t f", f=D_OUT),
                    in0=ps[:, :g * D_OUT]
                    .rearrange("p (t f) -> p t f", f=D_OUT),
                    in1=s1[:, t0:t1].to_broadcast([P, g, D_OUT]),
                    op=mybir.AluOpType.mult,
                )
                if t1 == SPLIT1:
                    nc.scalar.dma_start(
                        out=h1[:, :SPLIT1 * D_OUT], in_=h1a[:]
                    )
            nc.scalar.dma_start(out=h1[:, SPLIT1 * D_OUT:], in_=h1b[:])
    nc.finalize()
    return nc


# -------------------------------------------------------- pass-2 builder ---
def build_pass2(ktile, offs, totk):
    nc = bacc.Bacc(None, num_devices=NCORES)
    he = nc.dram_tensor("he", [P, totk * D_OUT], F16, kind="ExternalInput")
    dego = nc.dram_tensor("dego", [P, TPC], F16, kind="ExternalInput")
    biasd = nc.dram_tensor("bias", [P, D_OUT], F32, kind="ExternalInput")
    identd = nc.dram_tensor("ident", [P, P], F16, kind="ExternalInput")
    out = nc.dram_tensor("out", [P, TPC * D_OUT], F16, kind="ExternalOutput")

    # Tiny warmup chunk first (engines start early), then reverse tile
    # order (ktile ascends) in 8-tile chunks, draining on small chunks.
    chunks = [(0, 2)]
    hi = TPC
    for size in (8,) * 11 + (4, 4):
        lo = max(hi - size, 2)
        if hi > lo:
            chunks.append((lo, hi))
        hi = lo
    assert hi == 2
    maxcols = max(int(offs[t1] - offs[t0]) for (t0, t1) in chunks) * D_OUT

    with tile.TileContext(nc) as tc:
        with (
            tc.tile_pool(name="const", bufs=1) as cpool,
            tc.tile_pool(name="hein", bufs=4) as hpool,
            tc.tile_pool(name="t32", bufs=3) as pool32,
            tc.tile_pool(name="psum", bufs=8, space="PSUM") as ppool,
        ):
            ident = cpool.tile([P, P], F16)
            deg_sb = cpool.tile([P, TPC], F16)
            bias_sb = cpool.tile([P, D_OUT], F32)
            rtmp = cpool.tile([P, TPC], F32)
            s_own = cpool.tile([P, TPC], F32)
            sqd = cpool.tile([P, TPC], F32)
            binv = cpool.tile([P, TPC * D_OUT], F32)
            out_sb = cpool.tile([P, TPC * D_OUT], F16)

            nc.scalar.dma_start(out=ident[:], in_=identd[:, :])
            nc.scalar.dma_start(out=deg_sb[:], in_=dego[:, :])
            nc.scalar.dma_start(out=bias_sb[:], in_=biasd[:, :])
            nc.vector.reciprocal(out=rtmp[:], in_=deg_sb[:])
            nc.scalar.sqrt(out=s_own[:], in_=rtmp[:])
            # binv[p,t,f] = b[f] * sqrt(deg[p,t]); (agg + binv) * s = s*agg + b
            nc.vector.tensor_tensor(
                out=sqd[:], in0=deg_sb[:], in1=s_own[:],
                op=mybir.AluOpType.mult,
            )
            bias_bc = bass.AP(
                bias_sb[:].tensor, bias_sb[:].offset,
                [[D_OUT, P], [0, TPC], [1, D_OUT]],
            )
            nc.vector.tensor_tensor(
                out=binv[:].rearrange("p (t f) -> p t f", f=D_OUT),
                in0=bias_bc,
                in1=sqd[:].to_broadcast([P, TPC, D_OUT]),
                op=mybir.AluOpType.mult,
            )

            hcs = {}

            def ensure_dma(cj):
                if cj >= len(chunks) or cj in hcs:
                    return
                u0, u1 = chunks[cj]
                cc0 = int(offs[u0]) * D_OUT
                ccols = (int(offs[u1]) - int(offs[u0])) * D_OUT
                hc = hpool.tile([P, maxcols], F16, tag="hc")
                nc.sync.dma_start(out=hc[:, :ccols], in_=he[:, cc0:cc0 + ccols])
                hcs[cj] = hc

            # out flush boundaries: flush [b:) as soon as it is covered
            bounds = [TPC - 48, 18, 0]
            left = [TPC - bounds[0], bounds[0] - 18, 18]
            for ci, (t0, t1) in enumerate(chunks):
                ensure_dma(ci)
                ensure_dma(ci + 1)
                ensure_dma(ci + 2)
                g = t1 - t0
                hc = hcs.pop(ci)

                t32 = pool32.tile([P, MAXC2 * D_OUT], F32, tag="t32")
                for p0 in range(t0, t1, PGRP2):
                    p1 = min(p0 + PGRP2, t1)
                    pg = p1 - p0
                    ps = ppool.tile([P, PGRP2 * 4 * D_OUT], F32, tag="ps")
                    first = True
                    for t in range(p0, p1):
                        k = int(ktile[t])
                        base = (int(offs[t]) - int(offs[t0])) * D_OUT
                        o = (t - p0) * 4 * D_OUT
                        for kp in range(k // 4):
                            nc.tensor.matmul(
                                out=ps[:, o:o + 4 * D_OUT],
                                lhsT=ident[:],
                                rhs=hc[:, base + kp * 4 * D_OUT:
                                       base + (kp + 1) * 4 * D_OUT],
                                start=first,
                                stop=(t == p1 - 1 and k % 4 != 2
                                      and kp == k // 4 - 1),
                            )
                            first = False
                        if k % 4 == 2:
                            nc.tensor.matmul(
                                out=ps[:, o:o + 2 * D_OUT],
                                lhsT=ident[:],
                                rhs=hc[:, base + (k - 2) * D_OUT:
                                       base + k * D_OUT],
                                start=False,
                                stop=(t == p1 - 1),
                            )
                    # fold all 4 lanes in one XY reduce (every lane is
                    # matmul-written: K >= 4 gives each tile a full
                    # 128-col pack, so no stale-PSUM reads)
                    nc.vector.tensor_reduce(
                        out=t32[:, (p0 - t0) * D_OUT:(p1 - t0) * D_OUT]
                        .rearrange("p (t f) -> p t f", f=D_OUT),
                        in_=ps[:, :pg * 4 * D_OUT].rearrange(
                            "p (t ka kb f) -> p t f ka kb",
                            ka=2, kb=2, f=D_OUT,
                        ),
                        axis=mybir.AxisListType.XY,
                        op=mybir.AluOpType.add,
                    )
                # pre-add b*sqrt(deg)
                nc.vector.tensor_tensor(
                    out=t32[:, :g * D_OUT]
                    .rearrange("p (t f) -> p t f", f=D_OUT),
                    in0=t32[:, :g * D_OUT]
                    .rearrange("p (t f) -> p t f", f=D_OUT),
                    in1=binv[:, t0 * D_OUT:t1 * D_OUT]
                    .rearrange("p (t f) -> p t f", f=D_OUT),
                    op=mybir.AluOpType.add,
                )
                # dest scale + relu: ACT engine in steady state (parallel
                # with DVE folds), but DVE for the drain chunks — ACT's
                # ~400ns/tile serial chain would otherwise be the tail
                if ci >= len(chunks) - 2:
                    nc.vector.tensor_tensor(
                        out=t32[:, :g * D_OUT]
                        .rearrange("p (t f) -> p t f", f=D_OUT),
                        in0=t32[:, :g * D_OUT]
                        .rearrange("p (t f) -> p t f", f=D_OUT),
                        in1=s_own[:, t0:t1].to_broadcast([P, g, D_OUT]),
                        op=mybir.AluOpType.mult,
                    )
                    nc.vector.tensor_scalar(
                        out=out_sb[:, t0 * D_OUT:t1 * D_OUT],
                        in0=t32[:, :g * D_OUT], scalar1=0.0, scalar2=None,
                        op0=mybir.AluOpType.max,
                    )
                else:
                    for t in range(t0, t1):
                        nc.scalar.activation(
                            out=out_sb[:, t * D_OUT:(t + 1) * D_OUT],
                            in_=t32[:, (t - t0) * D_OUT:(t - t0 + 1) * D_OUT],
                            func=mybir.ActivationFunctionType.Relu,
                            scale=s_own[:, t:t + 1],
                        )
                for bi, b in enumerate(bounds):
                    blo = b
                    bhi = TPC if bi == 0 else bounds[bi - 1]
                    ov = max(0, min(t1, bhi) - max(t0, blo))
                    if ov and left[bi] > 0:
                        left[bi] -= ov
                        if left[bi] == 0:
                            nc.sync.dma_start(
                                out=out[:, blo * D_OUT:bhi * D_OUT],
                                in_=out_sb[:, blo * D_OUT:bhi * D_OUT],
                            )
    nc.finalize()
    return nc


# ---------------------------------------------------------------- runner ---
def _run(inputs, trace=False):
    x = inputs["x"]
    w16 = np.asarray(inputs["W"]).astype(np.float16)
    bias = np.broadcast_to(
        np.asarray(inputs["b"]).astype(np.float32), (P, D_OUT)
    ).copy()
    ident = np.eye(P, dtype=np.float16)

    xt, degp, src_slot, ktile, offs, totk, node_of_pos = host_prep(
        x, inputs["edge_index"]
    )

    nc1 = build_pass1()
    in1 = [
        {"xt": xt[c], "deg1": degp[c], "w": w16} for c in range(NCORES)
    ]
    res1 = bass_utils.run_bass_kernel_spmd(
        nc1, in1, core_ids=list(range(NCORES)), trace=trace
    )

    # assemble h' (augmented with a zero row for -1 slots), gather he
    h_aug = np.zeros((N + 1, D_OUT), dtype=np.float16)
    for c in range(NCORES):
        hc = res1.results[c]["h1"].reshape(P, TPC, D_OUT)
        block = hc.transpose(1, 0, 2).reshape(NPOS, D_OUT)
        nid = node_of_pos[c * NPOS:(c + 1) * NPOS]
        m = nid >= 0
        h_aug[nid[m]] = block[m]
    slot = np.where(src_slot >= 0, src_slot, N)

    nc2 = build_pass2(ktile, offs, totk)
    in2 = [
        {"he": h_aug[slot[c]].reshape(P, totk * D_OUT),
         "dego": degp[c], "bias": bias, "ident": ident}
        for c in range(NCORES)
    ]
    res2 = bass_utils.run_bass_kernel_spmd(
        nc2, in2, core_ids=list(range(NCORES)), trace=trace
    )

    full = np.empty((N, D_OUT), dtype=np.float32)
    for c in range(NCORES):
        oc = res2.results[c]["out"].reshape(P, TPC, D_OUT)
        block = oc.transpose(1, 0, 2).reshape(NPOS, D_OUT).astype(np.float32)
        nid = node_of_pos[c * NPOS:(c + 1) * NPOS]
        m = nid >= 0
        full[nid[m]] = block[m]
    return full, [res1, res2]


def kernel(**inputs) -> np.ndarray:
    full, _ = _run(inputs, trace=False)
    return full
